# revision 28
# baseline (speedup 1.0000x reference)
"""Trainium2 Bass kernel for nn_AttentionBlock (dense_cnn, memory-bound).

Computation (per reference):
    g1  = BN(gate @ Wg)            # biases cancel inside BN
    x1  = BN(skip @ Wx)
    psi = relu(g1 + x1)
    t   = psi @ Wpsi               # bpsi cancels inside BN
    out = skip * sigmoid(BN(t))

BN is training-mode batch-norm over the full point dim N=1e6, so the kernel
is a 3-pass pipeline per core with two tiny AllReduces:

  Phase A: stream gate+skip once (f32->bf16 cast in the SWDGE DMA
           datapath), PE-transpose 128x128 tiles to feat-major, compute
           z = [Wg^T gT ; Wx^T sT] stacked on 128 partitions (64 g-channels
           + 64 x-channels), store z to DRAM in bf16, and harvest
           per-channel mean/var with one DVE bn_stats per 512 columns
           (bn_aggr at the end; converted to raw sums for the AllReduce).
           bf16 input rounding is unbiased and averages out over 1e6 rows.
  AR1:     AllReduce of stacked [128,2] channel stats -> BN scale column
           a = gamma/sqrt(var+eps) folded into a double-diagonal matrix
           DD[c,c]=a_g[c], DD[64+c,c]=a_x[c] and a bias column c.
  Phase B: read z back (32 MB instead of re-reading 128 MB of inputs);
           psum_v = DD^T z  (one matmul per 512 rows),
           psi = ACT.relu(psum_v + c) -> bf16,
           t per tile via psi_tile^T @ Wpsi -> [128,1] columns that land
           directly in the [128, QT] per-partition t layout in SBUF.
  AR2:     sum/sumsq of t (pad rows analytically removed) -> sigmoid
           affine; s = sigmoid(a*t+b) kept in SBUF.
  Phase C: out = skip * s with per-partition tensor_scalar multiplies.

All phases use one row mapping row = p*QT + q (QT = rows/128), so every
large DMA moves >=5KB contiguous bytes per partition.  Sharding: rows
padded to 125,440/core * 8 cores; pad rows are zero, contribute zero to
all linear stats, and their constant t value is removed exactly via the
n_pad correction before AR2.
"""

import sys

for _p in ("/opt/trn_rl_repo", "/root/.axon_site/_ro/trn_rl_repo"):
    if _p not in sys.path:
        sys.path.insert(0, _p)

import numpy as np

from concourse import bacc, bass, mybir, tile
from concourse.bass_utils import run_bass_kernel_spmd

F32 = mybir.dt.float32
BF16 = mybir.dt.bfloat16
AF = mybir.ActivationFunctionType
ALU = mybir.AluOpType
AX = mybir.AxisListType

N_CORES = 8
N_TOTAL = 1_000_000
ROWS_PER_CORE = 125_440          # = 128 * 980
JQ = 20                          # q-columns per chunk; 980 = 49*20
EPS = 1e-5


def build_nc(rows=ROWS_PER_CORE, n_total=N_TOTAL, n_cores=N_CORES, jq=JQ):
    assert rows % 128 == 0
    qt = rows // 128
    assert qt % jq == 0 and jq % 4 == 0
    n_chunks = qt // jq
    n_subs = jq // 4                 # 512-column sub-blocks per chunk
    n_slots = n_chunks * n_subs
    inv_n = 1.0 / float(n_total)

    nc = bacc.Bacc("TRN2", target_bir_lowering=False, debug=False,
                   num_devices=n_cores)

    g_d = nc.dram_tensor("g", [rows, 128], F32, kind="ExternalInput").ap()
    s_d = nc.dram_tensor("s", [rows, 128], F32, kind="ExternalInput").ap()
    wg_d = nc.dram_tensor("wg", [128, 64], F32, kind="ExternalInput").ap()
    wx_d = nc.dram_tensor("wx", [128, 64], F32, kind="ExternalInput").ap()
    wpsi_d = nc.dram_tensor("wpsi", [64, 1], F32, kind="ExternalInput").ap()
    gstk_d = nc.dram_tensor("gstk", [128, 1], F32, kind="ExternalInput").ap()
    bstk_d = nc.dram_tensor("bstk", [128, 1], F32, kind="ExternalInput").ap()
    gam_p_d = nc.dram_tensor("gam_p", [1, 1], F32, kind="ExternalInput").ap()
    bet_p_d = nc.dram_tensor("bet_p", [1, 1], F32, kind="ExternalInput").ap()
    npad_d = nc.dram_tensor("npad", [1, 1], F32, kind="ExternalInput").ap()
    ident_d = nc.dram_tensor("ident", [128, 128], BF16, kind="ExternalInput").ap()
    e2_d = nc.dram_tensor("e2", [128, 64], F32, kind="ExternalInput").ap()
    onec_d = nc.dram_tensor("onec", [128, 1], F32, kind="ExternalInput").ap()
    oner_d = nc.dram_tensor("oner", [1, 128], F32, kind="ExternalInput").ap()
    out_d = nc.dram_tensor("out", [rows, 128], F32, kind="ExternalOutput").ap()

    # row mapping: row = p*qt + q   (partition-major; contiguous per partition)
    g_pq = g_d.rearrange("(p q) f -> p q f", p=128)
    s_pq = s_d.rearrange("(p q) f -> p q f", p=128)
    o_pq = out_d.rearrange("(p q) f -> p q f", p=128)

    with tile.TileContext(nc) as tc:
        with (
            tc.tile_pool(name="singles", bufs=1) as singles,
            tc.tile_pool(name="stats", bufs=1) as stats,
            tc.tile_pool(name="dram", bufs=1, space="DRAM") as dpool,
        ):
            # ---- constants to SBUF ----
            sb_wg = singles.tile([128, 64], F32, tag="wg")
            sb_wx = singles.tile([128, 64], F32, tag="wx")
            sb_wg_bf = singles.tile([128, 64], BF16, tag="wgb")
            sb_wx_bf = singles.tile([128, 64], BF16, tag="wxb")
            sb_wpsi = singles.tile([64, 1], F32, tag="wpsi")
            sb_wpsi_bf = singles.tile([64, 1], BF16, tag="wpsib")
            sb_ident = singles.tile([128, 128], BF16, tag="ident")
            sb_e2 = singles.tile([128, 64], F32, tag="e2")
            sb_onec = singles.tile([128, 1], F32, tag="onec")
            sb_oner = singles.tile([1, 128], F32, tag="oner")
            sb_gstk = singles.tile([128, 1], F32, tag="gstk")
            sb_bstk = singles.tile([128, 1], F32, tag="bstk")
            sb_gp = singles.tile([1, 1], F32, tag="gp")
            sb_bp = singles.tile([1, 1], F32, tag="bp")
            sb_npad = singles.tile([1, 1], F32, tag="npad")
            nc.sync.dma_start(out=sb_wg, in_=wg_d)
            nc.sync.dma_start(out=sb_wx, in_=wx_d)
            nc.sync.dma_start(out=sb_wpsi, in_=wpsi_d)
            nc.sync.dma_start(out=sb_ident, in_=ident_d)
            nc.sync.dma_start(out=sb_e2, in_=e2_d)
            nc.sync.dma_start(out=sb_onec, in_=onec_d)
            nc.sync.dma_start(out=sb_oner, in_=oner_d)
            nc.sync.dma_start(out=sb_gstk, in_=gstk_d)
            nc.sync.dma_start(out=sb_bstk, in_=bstk_d)
            nc.sync.dma_start(out=sb_gp, in_=gam_p_d)
            nc.sync.dma_start(out=sb_bp, in_=bet_p_d)
            nc.sync.dma_start(out=sb_npad, in_=npad_d)
            nc.vector.tensor_copy(sb_wpsi_bf, sb_wpsi)
            nc.vector.tensor_copy(sb_wg_bf, sb_wg)
            nc.vector.tensor_copy(sb_wx_bf, sb_wx)

            z_dram = dpool.tile([128, rows], BF16, tag="z")
            ar1_in = dpool.tile([128, 2], F32, tag="ar1i")
            ar1_out = dpool.tile([128, 2], F32, tag="ar1o")
            ar2_in = dpool.tile([1, 2], F32, tag="ar2i")
            ar2_out = dpool.tile([1, 2], F32, tag="ar2o")

            rg = [list(range(n_cores))]

            qtp = ((qt + 127) // 128) * 128   # xbar free-dim multiple of 128
            t_all = stats.tile([128, qtp], BF16, tag="tall")
            slots6 = stats.tile([128, n_slots, 6], F32, tag="slots6")
            t_dram = dpool.tile([qtp, 128], BF16, tag="t")

            # =========== Phase A: z = [Wg^T gT ; Wx^T sT], stats, store ====
            with (
                tc.tile_pool(name="pa", bufs=4) as pa,
                tc.tile_pool(name="psAt", bufs=4, space="PSUM") as psAt,
                tc.tile_pool(name="psA", bufs=2, space="PSUM") as psA,
            ):
                for ch in range(n_chunks):
                    q0 = ch * jq
                    gab = pa.tile([128, jq, 128], BF16, tag="gab")
                    xab = pa.tile([128, jq, 128], BF16, tag="xab")
                    nc.gpsimd.dma_start(out=gab, in_=g_pq[:, q0:q0 + jq, :])
                    nc.gpsimd.dma_start(out=xab, in_=s_pq[:, q0:q0 + jq, :])
                    zst = pa.tile([128, n_subs, 512], BF16, tag="zst")
                    for su in range(n_subs):
                        slot = ch * n_subs + su
                        pTg = psAt.tile([128, 512], BF16, tag="pT")
                        pTx = psAt.tile([128, 512], BF16, tag="pT")
                        for j in range(4):
                            k = su * 4 + j
                            nc.tensor.transpose(pTg[:, j * 128:(j + 1) * 128],
                                                gab[:, k, :], sb_ident)
                            nc.tensor.transpose(pTx[:, j * 128:(j + 1) * 128],
                                                xab[:, k, :], sb_ident)
                        gT = pa.tile([128, 512], BF16, tag="gT")
                        sT = pa.tile([128, 512], BF16, tag="sT")
                        nc.vector.tensor_copy(gT, pTg)
                        nc.scalar.copy(sT, pTx)
                        pz = psA.tile([128, 512], F32, tag="pz")
                        nc.tensor.matmul(pz[0:64, :], lhsT=sb_wg_bf, rhs=gT,
                                         start=True, stop=True)
                        nc.tensor.matmul(pz[64:128, :], lhsT=sb_wx_bf, rhs=sT,
                                         start=True, stop=True)
                        nc.scalar.copy(zst[:, su, :], pz)
                        nc.vector.bn_stats(slots6[:, slot, :], pz)
                    nc.sync.dma_start(
                        out=z_dram[:, q0 * 128:(q0 + jq) * 128],
                        in_=zst.rearrange("p s c -> p (s c)"))

                # ---- stacked channel stats -> AR1 ----
                # bn_aggr -> per-channel (mean, var) over this core's rows;
                # convert to raw sums so cores combine linearly.
                mv = stats.tile([128, 2], F32, tag="mv")
                nc.vector.bn_aggr(mv, slots6)
                ar1_sb = stats.tile([128, 2], F32, tag="ar1sb")
                msq = stats.tile([128, 1], F32, tag="msq")
                nc.vector.tensor_mul(msq, mv[:, 0:1], mv[:, 0:1])
                nc.vector.tensor_add(msq, msq, mv[:, 1:2])
                nc.scalar.mul(ar1_sb[:, 0:1], mv[:, 0:1], float(rows))
                nc.scalar.mul(ar1_sb[:, 1:2], msq, float(rows))
                nc.sync.dma_start(out=ar1_in, in_=ar1_sb)
                nc.gpsimd.collective_compute(
                    "AllReduce", ALU.add, replica_groups=rg,
                    ins=[ar1_in.opt()], outs=[ar1_out.opt()])
                sbStats = stats.tile([128, 2], F32, tag="sbStats")
                nc.sync.dma_start(out=sbStats, in_=ar1_out)

                # ---- BN affine (stacked [128,1] columns) ----
                mu_s = stats.tile([128, 1], F32, tag="mus")
                a_s = stats.tile([128, 1], F32, tag="as")
                colA = stats.tile([128, 1], F32, tag="colA")
                tmp1 = stats.tile([128, 1], F32, tag="tmp1")
                tmp2 = stats.tile([128, 1], F32, tag="tmp2")
                nc.scalar.mul(mu_s, sbStats[:, 0:1], inv_n)
                nc.scalar.mul(tmp1, sbStats[:, 1:2], inv_n)
                nc.vector.tensor_mul(tmp2, mu_s, mu_s)
                nc.vector.tensor_sub(tmp1, tmp1, tmp2)
                nc.vector.tensor_scalar_add(tmp1, tmp1, EPS)
                nc.scalar.activation(tmp1, tmp1, AF.Sqrt)
                nc.vector.reciprocal(tmp2, tmp1)
                nc.vector.tensor_mul(a_s, tmp2, sb_gstk)
                # colA = beta - mu*a   (stacked)
                nc.vector.tensor_mul(tmp1, mu_s, a_s)
                nc.vector.tensor_sub(colA, sb_bstk, tmp1)

                # DD = E2 * a_s (per-partition scalar), bf16
                dd_f = stats.tile([128, 64], F32, tag="ddf")
                dd_bf = stats.tile([128, 64], BF16, tag="ddb")
                nc.vector.tensor_scalar_mul(dd_f, sb_e2, a_s)
                nc.vector.tensor_copy(dd_bf, dd_f)

                # c_col[c] = colA[c] + colA[64+c]  via E2^T @ colA
                c_col = stats.tile([64, 1], F32, tag="ccol")
                pcc = psA.tile([64, 1], F32, tag="st")
                nc.tensor.matmul(pcc, lhsT=sb_e2, rhs=colA, start=True, stop=True)
                nc.vector.tensor_copy(c_col, pcc)

                # t value of an all-zero (pad) row: sum_c relu(c)_c * wpsi_c
                t_pad = stats.tile([1, 1], F32, tag="tpad")
                rcw = stats.tile([64, 1], F32, tag="rcw")
                nc.scalar.activation(rcw, c_col, AF.Relu)
                nc.vector.tensor_mul(rcw, rcw, sb_wpsi)
                ptp = psA.tile([1, 1], F32, tag="st")
                nc.tensor.matmul(ptp, lhsT=rcw, rhs=sb_onec[0:64, :],
                                 start=True, stop=True)
                nc.vector.tensor_copy(t_pad, ptp)

            # =========== Phase B: psi and t from stored z ===========
            with (
                tc.tile_pool(name="pbz", bufs=8) as pbz,
                tc.tile_pool(name="pb", bufs=3) as pb,
                tc.tile_pool(name="psB", bufs=4, space="PSUM") as psB,
            ):
                for ch in range(n_chunks):
                    q0 = ch * jq
                    zc = pbz.tile([128, jq * 128], BF16, tag="zc")
                    nc.sync.dma_start(out=zc,
                                      in_=z_dram[:, q0 * 128:(q0 + jq) * 128])
                    trow = pb.tile([1, jq, 128], F32, tag="trow")
                    for su in range(n_subs):
                        pv = psB.tile([64, 512], F32, tag="pv")
                        nc.tensor.matmul(pv, lhsT=dd_bf,
                                         rhs=zc[:, su * 512:(su + 1) * 512],
                                         start=True, stop=True)
                        psi = pb.tile([64, 512], BF16, tag="psi")
                        nc.scalar.activation(psi, pv, AF.Relu, bias=c_col)
                        ptr = psB.tile([1, 512], F32, tag="ptr")
                        nc.tensor.matmul(ptr, lhsT=sb_wpsi_bf, rhs=psi,
                                         start=True, stop=True)
                        nc.vector.tensor_copy(
                            trow[:, su * 4:(su + 1) * 4, :],
                            ptr.rearrange("o (q f) -> o q f", f=128))
                    # t stored (q, p)-ordered in bf16; read back transposed
                    nc.gpsimd.dma_start(
                        out=t_dram[q0:q0 + jq, :].unsqueeze(0), in_=trow)

            # =========== t statistics + AR2 ===========
            nc.sync.dma_start(out=t_all, in_=t_dram[:, :], transpose=True)
            tsums = stats.tile([128, 2], F32, tag="tsums")
            tsq = stats.tile([128, qt], F32, tag="tsq")
            nc.vector.tensor_reduce(tsums[:, 0:1], t_all[:, 0:qt],
                                    axis=AX.X, op=ALU.add)
            nc.vector.tensor_mul(tsq, t_all[:, 0:qt], t_all[:, 0:qt])
            nc.vector.tensor_reduce(tsums[:, 1:2], tsq, axis=AX.X, op=ALU.add)

            with tc.tile_pool(name="psT", bufs=1, space="PSUM") as psT:
                pr = psT.tile([1, 2], F32, tag="pr")
                nc.tensor.matmul(pr, lhsT=sb_onec, rhs=tsums, start=True, stop=True)
                # subtract pad-row contribution: npad * t_pad, npad * t_pad^2
                ar2_sb = stats.tile([1, 2], F32, tag="ar2sb")
                tp2 = stats.tile([1, 1], F32, tag="tp2")
                corr = stats.tile([1, 2], F32, tag="corr")
                nc.vector.tensor_mul(tp2, t_pad, t_pad)
                nc.vector.tensor_mul(corr[:, 0:1], sb_npad, t_pad)
                nc.vector.tensor_mul(corr[:, 1:2], sb_npad, tp2)
                nc.vector.tensor_sub(ar2_sb, pr, corr)
                nc.sync.dma_start(out=ar2_in, in_=ar2_sb)
                nc.gpsimd.collective_compute(
                    "AllReduce", ALU.add, replica_groups=rg,
                    ins=[ar2_in.opt()], outs=[ar2_out.opt()])
                sbT = stats.tile([1, 2], F32, tag="sbT")
                nc.sync.dma_start(out=sbT, in_=ar2_out)

                # sigmoid affine: a_p = gam_p/sqrt(var+eps), b = beta_p - mu*a_p
                mu_t = stats.tile([1, 1], F32, tag="mut")
                a_p = stats.tile([1, 1], F32, tag="apsi")
                b_p = stats.tile([1, 1], F32, tag="bpsi")
                t1 = stats.tile([1, 1], F32, tag="t1")
                t2 = stats.tile([1, 1], F32, tag="t2")
                nc.scalar.mul(mu_t, sbT[:, 0:1], inv_n)
                nc.scalar.mul(t1, sbT[:, 1:2], inv_n)
                nc.vector.tensor_mul(t2, mu_t, mu_t)
                nc.vector.tensor_sub(t1, t1, t2)
                nc.vector.tensor_scalar_add(t1, t1, EPS)
                nc.scalar.activation(t1, t1, AF.Sqrt)
                nc.vector.reciprocal(t2, t1)
                nc.vector.tensor_mul(a_p, t2, sb_gp)
                nc.vector.tensor_mul(t1, mu_t, a_p)
                nc.vector.tensor_sub(b_p, sb_bp, t1)

                # broadcast a_p, b_p to [128,1] columns
                ap_col = stats.tile([128, 1], F32, tag="apcol")
                bp_col = stats.tile([128, 1], F32, tag="bpcol")
                pb1 = psT.tile([128, 1], F32, tag="pb1")
                nc.tensor.matmul(pb1, lhsT=sb_oner, rhs=a_p, start=True, stop=True)
                nc.vector.tensor_copy(ap_col, pb1)
                pb2 = psT.tile([128, 1], F32, tag="pb2")
                nc.tensor.matmul(pb2, lhsT=sb_oner, rhs=b_p, start=True, stop=True)
                nc.vector.tensor_copy(bp_col, pb2)

            s_gate = stats.tile([128, qt], F32, tag="sgate")
            nc.scalar.activation(s_gate, t_all[:, 0:qt], AF.Sigmoid,
                                 bias=bp_col, scale=ap_col)

            # =========== Phase C: out = skip * s ===========
            with tc.tile_pool(name="pc", bufs=5) as pc:
                for b in range(n_chunks):
                    q0 = b * jq
                    sc = pc.tile([128, jq, 128], F32, tag="sc")
                    oc = pc.tile([128, jq, 128], F32, tag="oc")
                    nc.sync.dma_start(out=sc, in_=s_pq[:, q0:q0 + jq, :])
                    sg = (s_gate[:, q0:q0 + jq].unsqueeze(-1)
                          .broadcast_to([128, jq, 128]))
                    nc.vector.tensor_mul(oc, sc, sg)
                    nc.sync.dma_start(out=o_pq[:, q0:q0 + jq, :], in_=oc)

    nc.compile()
    return nc


def _in_maps(gate, skip, Wg, Wx, Wpsi, gamma_g, beta_g, gamma_x, beta_x,
             gamma_psi, beta_psi, rows, n_cores):
    import ml_dtypes
    n = gate.shape[0]
    total = rows * n_cores
    gp = np.zeros((total, 128), np.float32)
    sp = np.zeros((total, 128), np.float32)
    gp[:n] = gate
    sp[:n] = skip
    gstk = np.concatenate([np.asarray(gamma_g, np.float32).ravel(),
                           np.asarray(gamma_x, np.float32).ravel()])
    bstk = np.concatenate([np.asarray(beta_g, np.float32).ravel(),
                           np.asarray(beta_x, np.float32).ravel()])
    eye64 = np.eye(64, dtype=np.float32)
    common = {
        "wg": np.ascontiguousarray(Wg, np.float32),
        "wx": np.ascontiguousarray(Wx, np.float32),
        "wpsi": np.ascontiguousarray(Wpsi, np.float32).reshape(64, 1),
        "gstk": gstk.reshape(128, 1),
        "bstk": bstk.reshape(128, 1),
        "gam_p": np.asarray(gamma_psi, np.float32).reshape(1, 1),
        "bet_p": np.asarray(beta_psi, np.float32).reshape(1, 1),
        "ident": np.eye(128).astype(ml_dtypes.bfloat16),
        "e2": np.vstack([eye64, eye64]),
        "onec": np.ones((128, 1), np.float32),
        "oner": np.ones((1, 128), np.float32),
    }
    maps = []
    for i in range(n_cores):
        lo, hi = i * rows, (i + 1) * rows
        n_pad = hi - min(max(n, lo), hi)
        m = dict(common)
        m["g"] = gp[lo:hi]
        m["s"] = sp[lo:hi]
        m["npad"] = np.full((1, 1), float(n_pad), np.float32)
        maps.append(m)
    return maps


_NC_CACHE = {}


def kernel(gate, skip_connection, Wg, bg, gamma_g, beta_g,
           Wx, bx, gamma_x, beta_x, Wpsi, bpsi, gamma_psi, beta_psi,
           _trace=False):
    gate = np.asarray(gate, np.float32)
    skip = np.asarray(skip_connection, np.float32)
    n = gate.shape[0]

    key = (ROWS_PER_CORE, n, N_CORES)
    if key not in _NC_CACHE:
        _NC_CACHE[key] = build_nc(rows=ROWS_PER_CORE, n_total=n,
                                  n_cores=N_CORES)
    nc = _NC_CACHE[key]

    maps = _in_maps(gate, skip, Wg, Wx, Wpsi, gamma_g, beta_g,
                    gamma_x, beta_x, gamma_psi, beta_psi,
                    ROWS_PER_CORE, N_CORES)
    res = run_bass_kernel_spmd(nc, maps, core_ids=list(range(N_CORES)),
                               trace=_trace)
    out = np.concatenate([res.results[i]["out"] for i in range(N_CORES)],
                         axis=0)[:n]
    if _trace:
        kernel.last_results = res
    return out


# revision 31
# speedup vs baseline: 1.0621x; 1.0621x over previous
"""Trainium2 Bass kernel for nn_AttentionBlock (dense_cnn, memory-bound).

Computation (per reference):
    g1  = BN(gate @ Wg)            # biases cancel inside BN
    x1  = BN(skip @ Wx)
    psi = relu(g1 + x1)
    t   = psi @ Wpsi               # bpsi cancels inside BN
    out = skip * sigmoid(BN(t))

BN is training-mode batch-norm over the full point dim N=1e6, so the kernel
is a 3-pass pipeline per core with two tiny AllReduces:

  Phase A: stream gate+skip once (f32->bf16 cast in the SWDGE DMA
           datapath), PE-transpose 128x128 tiles to feat-major, compute
           z = [Wg^T gT ; Wx^T sT] stacked on 128 partitions (64 g-channels
           + 64 x-channels), store z to DRAM in bf16, and harvest
           per-channel mean/var with one DVE bn_stats per 512 columns
           (bn_aggr at the end; converted to raw sums for the AllReduce).
           bf16 input rounding is unbiased and averages out over 1e6 rows.
  AR1:     AllReduce of stacked [128,2] channel stats -> BN scale column
           a = gamma/sqrt(var+eps) folded into a double-diagonal matrix
           DD[c,c]=a_g[c], DD[64+c,c]=a_x[c] and a bias column c.
  Phase B: read z back (32 MB instead of re-reading 128 MB of inputs);
           psum_v = DD^T z  (one matmul per 512 rows),
           psi = ACT.relu(psum_v + c) -> bf16,
           t per tile via psi_tile^T @ Wpsi -> [128,1] columns that land
           directly in the [128, QT] per-partition t layout in SBUF.
  AR2:     sum/sumsq of t (pad rows analytically removed) -> sigmoid
           affine; s = sigmoid(a*t+b) kept in SBUF.
  Phase C: out = skip * s with per-partition tensor_scalar multiplies.

All phases use one row mapping row = p*QT + q (QT = rows/128), so every
large DMA moves >=5KB contiguous bytes per partition.  Sharding: rows
padded to 125,440/core * 8 cores; pad rows are zero, contribute zero to
all linear stats, and their constant t value is removed exactly via the
n_pad correction before AR2.
"""

import sys

for _p in ("/opt/trn_rl_repo", "/root/.axon_site/_ro/trn_rl_repo"):
    if _p not in sys.path:
        sys.path.insert(0, _p)

import numpy as np

from concourse import bacc, bass, mybir, tile
from concourse.bass_utils import run_bass_kernel_spmd

F32 = mybir.dt.float32
BF16 = mybir.dt.bfloat16
AF = mybir.ActivationFunctionType
ALU = mybir.AluOpType
AX = mybir.AxisListType

N_CORES = 8
N_TOTAL = 1_000_000
ROWS_PER_CORE = 125_440          # = 128 * 980
JQ = 20                          # q-columns per chunk; 980 = 49*20
EPS = 1e-5


def build_nc(rows=ROWS_PER_CORE, n_total=N_TOTAL, n_cores=N_CORES, jq=JQ):
    assert rows % 128 == 0
    qt = rows // 128
    assert qt % jq == 0 and jq % 4 == 0
    n_chunks = qt // jq
    n_subs = jq // 4                 # 512-column sub-blocks per chunk
    n_slots = n_chunks * n_subs
    inv_n = 1.0 / float(n_total)

    nc = bacc.Bacc("TRN2", target_bir_lowering=False, debug=False,
                   num_devices=n_cores)

    g_d = nc.dram_tensor("g", [rows, 128], F32, kind="ExternalInput").ap()
    s_d = nc.dram_tensor("s", [rows, 128], F32, kind="ExternalInput").ap()
    wg_d = nc.dram_tensor("wg", [128, 64], F32, kind="ExternalInput").ap()
    wx_d = nc.dram_tensor("wx", [128, 64], F32, kind="ExternalInput").ap()
    wpsi_d = nc.dram_tensor("wpsi", [64, 1], F32, kind="ExternalInput").ap()
    gstk_d = nc.dram_tensor("gstk", [128, 1], F32, kind="ExternalInput").ap()
    bstk_d = nc.dram_tensor("bstk", [128, 1], F32, kind="ExternalInput").ap()
    gam_p_d = nc.dram_tensor("gam_p", [1, 1], F32, kind="ExternalInput").ap()
    bet_p_d = nc.dram_tensor("bet_p", [1, 1], F32, kind="ExternalInput").ap()
    npad_d = nc.dram_tensor("npad", [1, 1], F32, kind="ExternalInput").ap()
    ident_d = nc.dram_tensor("ident", [128, 128], BF16, kind="ExternalInput").ap()
    e2_d = nc.dram_tensor("e2", [128, 64], F32, kind="ExternalInput").ap()
    onec_d = nc.dram_tensor("onec", [128, 1], F32, kind="ExternalInput").ap()
    oner_d = nc.dram_tensor("oner", [1, 128], F32, kind="ExternalInput").ap()
    out_d = nc.dram_tensor("out", [rows, 128], F32, kind="ExternalOutput").ap()

    # row mapping: row = p*qt + q   (partition-major; contiguous per partition)
    g_pq = g_d.rearrange("(p q) f -> p q f", p=128)
    s_pq = s_d.rearrange("(p q) f -> p q f", p=128)
    o_pq = out_d.rearrange("(p q) f -> p q f", p=128)

    with tile.TileContext(nc) as tc:
        with (
            tc.tile_pool(name="singles", bufs=1) as singles,
            tc.tile_pool(name="stats", bufs=1) as stats,
            tc.tile_pool(name="dram", bufs=1, space="DRAM") as dpool,
        ):
            # ---- constants to SBUF ----
            sb_wg = singles.tile([128, 64], F32, tag="wg")
            sb_wx = singles.tile([128, 64], F32, tag="wx")
            sb_wg_bf = singles.tile([128, 64], BF16, tag="wgb")
            sb_wx_bf = singles.tile([128, 64], BF16, tag="wxb")
            sb_wpsi = singles.tile([64, 1], F32, tag="wpsi")
            sb_wpsi_bf = singles.tile([64, 1], BF16, tag="wpsib")
            sb_ident = singles.tile([128, 128], BF16, tag="ident")
            sb_e2 = singles.tile([128, 64], F32, tag="e2")
            sb_onec = singles.tile([128, 1], F32, tag="onec")
            sb_oner = singles.tile([1, 128], F32, tag="oner")
            sb_gstk = singles.tile([128, 1], F32, tag="gstk")
            sb_bstk = singles.tile([128, 1], F32, tag="bstk")
            sb_gp = singles.tile([1, 1], F32, tag="gp")
            sb_bp = singles.tile([1, 1], F32, tag="bp")
            sb_npad = singles.tile([1, 1], F32, tag="npad")
            nc.sync.dma_start(out=sb_wg, in_=wg_d)
            nc.sync.dma_start(out=sb_wx, in_=wx_d)
            nc.sync.dma_start(out=sb_wpsi, in_=wpsi_d)
            nc.sync.dma_start(out=sb_ident, in_=ident_d)
            nc.sync.dma_start(out=sb_e2, in_=e2_d)
            nc.sync.dma_start(out=sb_onec, in_=onec_d)
            nc.sync.dma_start(out=sb_oner, in_=oner_d)
            nc.sync.dma_start(out=sb_gstk, in_=gstk_d)
            nc.sync.dma_start(out=sb_bstk, in_=bstk_d)
            nc.sync.dma_start(out=sb_gp, in_=gam_p_d)
            nc.sync.dma_start(out=sb_bp, in_=bet_p_d)
            nc.sync.dma_start(out=sb_npad, in_=npad_d)
            nc.vector.tensor_copy(sb_wpsi_bf, sb_wpsi)
            nc.vector.tensor_copy(sb_wg_bf, sb_wg)
            nc.vector.tensor_copy(sb_wx_bf, sb_wx)

            z_dram = dpool.tile([128, rows], BF16, tag="z")
            ar1_in = dpool.tile([128, 2], F32, tag="ar1i")
            ar1_out = dpool.tile([128, 2], F32, tag="ar1o")
            ar2_in = dpool.tile([1, 2], F32, tag="ar2i")
            ar2_out = dpool.tile([1, 2], F32, tag="ar2o")

            rg = [list(range(n_cores))]

            t_all = stats.tile([128, qt], F32, tag="tall")
            slots6 = stats.tile([128, n_slots, 6], F32, tag="slots6")

            # =========== Phase A: z = [Wg^T gT ; Wx^T sT], stats, store ====
            with (
                tc.tile_pool(name="pa", bufs=4) as pa,
                tc.tile_pool(name="psAt", bufs=4, space="PSUM") as psAt,
                tc.tile_pool(name="psA", bufs=2, space="PSUM") as psA,
            ):
                for ch in range(n_chunks):
                    q0 = ch * jq
                    gab = pa.tile([128, jq, 128], BF16, tag="gab")
                    xab = pa.tile([128, jq, 128], BF16, tag="xab")
                    nc.gpsimd.dma_start(out=gab, in_=g_pq[:, q0:q0 + jq, :])
                    nc.gpsimd.dma_start(out=xab, in_=s_pq[:, q0:q0 + jq, :])
                    zst = pa.tile([128, n_subs, 512], BF16, tag="zst")
                    for su in range(n_subs):
                        slot = ch * n_subs + su
                        pTg = psAt.tile([128, 512], BF16, tag="pT")
                        pTx = psAt.tile([128, 512], BF16, tag="pT")
                        for j in range(4):
                            k = su * 4 + j
                            nc.tensor.transpose(pTg[:, j * 128:(j + 1) * 128],
                                                gab[:, k, :], sb_ident)
                            nc.tensor.transpose(pTx[:, j * 128:(j + 1) * 128],
                                                xab[:, k, :], sb_ident)
                        gT = pa.tile([128, 512], BF16, tag="gT")
                        sT = pa.tile([128, 512], BF16, tag="sT")
                        nc.vector.tensor_copy(gT, pTg)
                        nc.scalar.copy(sT, pTx)
                        pz = psA.tile([128, 512], F32, tag="pz")
                        nc.tensor.matmul(pz[0:64, :], lhsT=sb_wg_bf, rhs=gT,
                                         start=True, stop=True)
                        nc.tensor.matmul(pz[64:128, :], lhsT=sb_wx_bf, rhs=sT,
                                         start=True, stop=True)
                        nc.scalar.copy(zst[:, su, :], pz)
                        nc.vector.bn_stats(slots6[:, slot, :], pz)
                    nc.sync.dma_start(
                        out=z_dram[:, q0 * 128:(q0 + jq) * 128],
                        in_=zst.rearrange("p s c -> p (s c)"))

                # ---- stacked channel stats -> AR1 ----
                # bn_aggr -> per-channel (mean, var) over this core's rows;
                # convert to raw sums so cores combine linearly.
                mv = stats.tile([128, 2], F32, tag="mv")
                nc.vector.bn_aggr(mv, slots6)
                ar1_sb = stats.tile([128, 2], F32, tag="ar1sb")
                msq = stats.tile([128, 1], F32, tag="msq")
                nc.vector.tensor_mul(msq, mv[:, 0:1], mv[:, 0:1])
                nc.vector.tensor_add(msq, msq, mv[:, 1:2])
                nc.scalar.mul(ar1_sb[:, 0:1], mv[:, 0:1], float(rows))
                nc.scalar.mul(ar1_sb[:, 1:2], msq, float(rows))
                nc.sync.dma_start(out=ar1_in, in_=ar1_sb)
                nc.gpsimd.collective_compute(
                    "AllReduce", ALU.add, replica_groups=rg,
                    ins=[ar1_in.opt()], outs=[ar1_out.opt()])
                sbStats = stats.tile([128, 2], F32, tag="sbStats")
                nc.sync.dma_start(out=sbStats, in_=ar1_out)

                # ---- BN affine (stacked [128,1] columns) ----
                mu_s = stats.tile([128, 1], F32, tag="mus")
                a_s = stats.tile([128, 1], F32, tag="as")
                colA = stats.tile([128, 1], F32, tag="colA")
                tmp1 = stats.tile([128, 1], F32, tag="tmp1")
                tmp2 = stats.tile([128, 1], F32, tag="tmp2")
                nc.scalar.mul(mu_s, sbStats[:, 0:1], inv_n)
                nc.scalar.mul(tmp1, sbStats[:, 1:2], inv_n)
                nc.vector.tensor_mul(tmp2, mu_s, mu_s)
                nc.vector.tensor_sub(tmp1, tmp1, tmp2)
                nc.vector.tensor_scalar_add(tmp1, tmp1, EPS)
                nc.scalar.activation(tmp1, tmp1, AF.Sqrt)
                nc.vector.reciprocal(tmp2, tmp1)
                nc.vector.tensor_mul(a_s, tmp2, sb_gstk)
                # colA = beta - mu*a   (stacked)
                nc.vector.tensor_mul(tmp1, mu_s, a_s)
                nc.vector.tensor_sub(colA, sb_bstk, tmp1)

                # DD = E2 * a_s (per-partition scalar), bf16
                dd_f = stats.tile([128, 64], F32, tag="ddf")
                dd_bf = stats.tile([128, 64], BF16, tag="ddb")
                nc.vector.tensor_scalar_mul(dd_f, sb_e2, a_s)
                nc.vector.tensor_copy(dd_bf, dd_f)

                # c_col[c] = colA[c] + colA[64+c]  via E2^T @ colA
                c_col = stats.tile([64, 1], F32, tag="ccol")
                pcc = psA.tile([64, 1], F32, tag="st")
                nc.tensor.matmul(pcc, lhsT=sb_e2, rhs=colA, start=True, stop=True)
                nc.vector.tensor_copy(c_col, pcc)

                # t value of an all-zero (pad) row: sum_c relu(c)_c * wpsi_c
                t_pad = stats.tile([1, 1], F32, tag="tpad")
                rcw = stats.tile([64, 1], F32, tag="rcw")
                nc.scalar.activation(rcw, c_col, AF.Relu)
                nc.vector.tensor_mul(rcw, rcw, sb_wpsi)
                ptp = psA.tile([1, 1], F32, tag="st")
                nc.tensor.matmul(ptp, lhsT=rcw, rhs=sb_onec[0:64, :],
                                 start=True, stop=True)
                nc.vector.tensor_copy(t_pad, ptp)

            # =========== Phase B: psi and t from stored z ===========
            with (
                tc.tile_pool(name="pbz", bufs=8) as pbz,
                tc.tile_pool(name="pb", bufs=3) as pb,
                tc.tile_pool(name="psB", bufs=4, space="PSUM") as psB,
            ):
                for ch in range(n_chunks):
                    q0 = ch * jq
                    zc = pbz.tile([128, jq * 128], BF16, tag="zc")
                    nc.sync.dma_start(out=zc,
                                      in_=z_dram[:, q0 * 128:(q0 + jq) * 128])
                    pt = psB.tile([128, jq], F32, tag="pt")
                    for su in range(n_subs):
                        pv = psB.tile([64, 512], F32, tag="pv")
                        nc.tensor.matmul(pv, lhsT=dd_bf,
                                         rhs=zc[:, su * 512:(su + 1) * 512],
                                         start=True, stop=True)
                        psi = pb.tile([64, 512], BF16, tag="psi")
                        nc.scalar.activation(psi, pv, AF.Relu, bias=c_col)
                        for j in range(4):
                            k = su * 4 + j
                            nc.tensor.matmul(pt[:, k:k + 1],
                                             lhsT=psi[:, j * 128:(j + 1) * 128],
                                             rhs=sb_wpsi_bf,
                                             start=True, stop=True)
                    nc.vector.tensor_copy(t_all[:, q0:q0 + jq], pt)

            # =========== t statistics + AR2 ===========
            tsums = stats.tile([128, 2], F32, tag="tsums")
            tsq = stats.tile([128, qt], F32, tag="tsq")
            nc.vector.tensor_reduce(tsums[:, 0:1], t_all, axis=AX.X, op=ALU.add)
            nc.vector.tensor_mul(tsq, t_all, t_all)
            nc.vector.tensor_reduce(tsums[:, 1:2], tsq, axis=AX.X, op=ALU.add)

            with tc.tile_pool(name="psT", bufs=1, space="PSUM") as psT:
                pr = psT.tile([1, 2], F32, tag="pr")
                nc.tensor.matmul(pr, lhsT=sb_onec, rhs=tsums, start=True, stop=True)
                # subtract pad-row contribution: npad * t_pad, npad * t_pad^2
                ar2_sb = stats.tile([1, 2], F32, tag="ar2sb")
                tp2 = stats.tile([1, 1], F32, tag="tp2")
                corr = stats.tile([1, 2], F32, tag="corr")
                nc.vector.tensor_mul(tp2, t_pad, t_pad)
                nc.vector.tensor_mul(corr[:, 0:1], sb_npad, t_pad)
                nc.vector.tensor_mul(corr[:, 1:2], sb_npad, tp2)
                nc.vector.tensor_sub(ar2_sb, pr, corr)
                nc.sync.dma_start(out=ar2_in, in_=ar2_sb)
                nc.gpsimd.collective_compute(
                    "AllReduce", ALU.add, replica_groups=rg,
                    ins=[ar2_in.opt()], outs=[ar2_out.opt()])
                sbT = stats.tile([1, 2], F32, tag="sbT")
                nc.sync.dma_start(out=sbT, in_=ar2_out)

                # sigmoid affine: a_p = gam_p/sqrt(var+eps), b = beta_p - mu*a_p
                mu_t = stats.tile([1, 1], F32, tag="mut")
                a_p = stats.tile([1, 1], F32, tag="apsi")
                b_p = stats.tile([1, 1], F32, tag="bpsi")
                t1 = stats.tile([1, 1], F32, tag="t1")
                t2 = stats.tile([1, 1], F32, tag="t2")
                nc.scalar.mul(mu_t, sbT[:, 0:1], inv_n)
                nc.scalar.mul(t1, sbT[:, 1:2], inv_n)
                nc.vector.tensor_mul(t2, mu_t, mu_t)
                nc.vector.tensor_sub(t1, t1, t2)
                nc.vector.tensor_scalar_add(t1, t1, EPS)
                nc.scalar.activation(t1, t1, AF.Sqrt)
                nc.vector.reciprocal(t2, t1)
                nc.vector.tensor_mul(a_p, t2, sb_gp)
                nc.vector.tensor_mul(t1, mu_t, a_p)
                nc.vector.tensor_sub(b_p, sb_bp, t1)

                # broadcast a_p, b_p to [128,1] columns
                ap_col = stats.tile([128, 1], F32, tag="apcol")
                bp_col = stats.tile([128, 1], F32, tag="bpcol")
                pb1 = psT.tile([128, 1], F32, tag="pb1")
                nc.tensor.matmul(pb1, lhsT=sb_oner, rhs=a_p, start=True, stop=True)
                nc.vector.tensor_copy(ap_col, pb1)
                pb2 = psT.tile([128, 1], F32, tag="pb2")
                nc.tensor.matmul(pb2, lhsT=sb_oner, rhs=b_p, start=True, stop=True)
                nc.vector.tensor_copy(bp_col, pb2)

            s_gate = stats.tile([128, qt], F32, tag="sgate")
            nc.scalar.activation(s_gate, t_all, AF.Sigmoid,
                                 bias=bp_col, scale=ap_col)

            # =========== Phase C: out = skip * s ===========
            with tc.tile_pool(name="pc", bufs=5) as pc:
                for b in range(n_chunks):
                    q0 = b * jq
                    sc = pc.tile([128, jq, 128], F32, tag="sc")
                    oc = pc.tile([128, jq, 128], F32, tag="oc")
                    nc.sync.dma_start(out=sc, in_=s_pq[:, q0:q0 + jq, :])
                    sg = (s_gate[:, q0:q0 + jq].unsqueeze(-1)
                          .broadcast_to([128, jq, 128]))
                    nc.vector.tensor_mul(oc, sc, sg)
                    nc.sync.dma_start(out=o_pq[:, q0:q0 + jq, :], in_=oc)

    nc.compile()
    return nc


def _in_maps(gate, skip, Wg, Wx, Wpsi, gamma_g, beta_g, gamma_x, beta_x,
             gamma_psi, beta_psi, rows, n_cores):
    import ml_dtypes
    n = gate.shape[0]
    total = rows * n_cores
    gp = np.zeros((total, 128), np.float32)
    sp = np.zeros((total, 128), np.float32)
    gp[:n] = gate
    sp[:n] = skip
    gstk = np.concatenate([np.asarray(gamma_g, np.float32).ravel(),
                           np.asarray(gamma_x, np.float32).ravel()])
    bstk = np.concatenate([np.asarray(beta_g, np.float32).ravel(),
                           np.asarray(beta_x, np.float32).ravel()])
    eye64 = np.eye(64, dtype=np.float32)
    common = {
        "wg": np.ascontiguousarray(Wg, np.float32),
        "wx": np.ascontiguousarray(Wx, np.float32),
        "wpsi": np.ascontiguousarray(Wpsi, np.float32).reshape(64, 1),
        "gstk": gstk.reshape(128, 1),
        "bstk": bstk.reshape(128, 1),
        "gam_p": np.asarray(gamma_psi, np.float32).reshape(1, 1),
        "bet_p": np.asarray(beta_psi, np.float32).reshape(1, 1),
        "ident": np.eye(128).astype(ml_dtypes.bfloat16),
        "e2": np.vstack([eye64, eye64]),
        "onec": np.ones((128, 1), np.float32),
        "oner": np.ones((1, 128), np.float32),
    }
    maps = []
    for i in range(n_cores):
        lo, hi = i * rows, (i + 1) * rows
        n_pad = hi - min(max(n, lo), hi)
        m = dict(common)
        m["g"] = gp[lo:hi]
        m["s"] = sp[lo:hi]
        m["npad"] = np.full((1, 1), float(n_pad), np.float32)
        maps.append(m)
    return maps


_NC_CACHE = {}


def kernel(gate, skip_connection, Wg, bg, gamma_g, beta_g,
           Wx, bx, gamma_x, beta_x, Wpsi, bpsi, gamma_psi, beta_psi,
           _trace=False):
    gate = np.asarray(gate, np.float32)
    skip = np.asarray(skip_connection, np.float32)
    n = gate.shape[0]

    key = (ROWS_PER_CORE, n, N_CORES)
    if key not in _NC_CACHE:
        _NC_CACHE[key] = build_nc(rows=ROWS_PER_CORE, n_total=n,
                                  n_cores=N_CORES)
    nc = _NC_CACHE[key]

    maps = _in_maps(gate, skip, Wg, Wx, Wpsi, gamma_g, beta_g,
                    gamma_x, beta_x, gamma_psi, beta_psi,
                    ROWS_PER_CORE, N_CORES)
    res = run_bass_kernel_spmd(nc, maps, core_ids=list(range(N_CORES)),
                               trace=_trace)
    out = np.concatenate([res.results[i]["out"] for i in range(N_CORES)],
                         axis=0)[:n]
    if _trace:
        kernel.last_results = res
    return out
